# revision 32
# baseline (speedup 1.0000x reference)
"""MMoE-style CustomizedGateControl kernel for 8x TRN2 NeuronCores.

Data-parallel over the batch dim (16384 -> 8 x 2048). Per core, expert
GEMMs run weights-stationary streaming batch columns, producing outputs
directly in [e, b] layout:
  - 12 expert GEMMs as 24 column-blocks (eb) of 128 e-rows x 2048 b-cols,
    PSUM [128, 2048] (4 banks) double-buffered, k-outer loop order so the
    walrus ldw-opt pass can elide redundant weight loads
  - drain = ONE fused ACT op per eb: relu(psum + bias[e]) -> fp16 SBUF
    (bias is per-partition in this layout)
  - gates [16, 2048] via tiny PE GEMM, broadcast to [128, 16, 2048] with
    partition-replicating DMA reads from a DRAM bounce buffer
  - gated combine on DVE only (GpSimd elementwise would mutually block
    with DVE on the shared SBUF port pair); shared experts feed both
    tasks in one paired [128, 2, 2048] op
  - tower MLPs read the [128, T, 2048] accumulators directly
All parameters replicated; no collectives.
"""

import sys

if "/opt/trn_rl_repo" not in sys.path:
    sys.path.insert(0, "/opt/trn_rl_repo")

import numpy as np

import concourse.bacc as bacc
import concourse.mybir as mybir
import concourse.tile as tile
import concourse.bass_utils as _bu
from concourse.bass_utils import run_bass_kernel_spmd

# problem dims
B, D, E, H = 16384, 512, 256, 128
S, K, T = 4, 4, 2
NCORES = 8
BC = B // NCORES          # 2048 batch rows per core
P = 128                   # partitions
KC = D // P               # 4 contraction chunks
NE = S + T * K            # 12 experts
G = S + K                 # 8 gate inputs per task
TG = T * G                # 16 gate columns
NEB = NE * 2              # 24 expert-output blocks of 128 e-rows
WCOLS = NE * E            # 3072 expert output columns
WALL = WCOLS + TG         # 3088 = experts + gate columns
NBK = BC // 512           # 4 batch chunks of 512 (psum bank size)

f32 = mybir.dt.float32
f16 = mybir.dt.float16

# sweep order: all half-0 blocks first, then half-1 (logical eb = expert*2+h)
SWEEP = [e * 2 for e in range(NE)] + [e * 2 + 1 for e in range(NE)]


def _build():
    nc = bacc.Bacc("TRN2", target_bir_lowering=False, debug=False)

    xt_d = nc.dram_tensor("xt", [D, BC], f16, kind="ExternalInput").ap()
    wall_d = nc.dram_tensor("wall", [D, WALL], f16, kind="ExternalInput").ap()
    biasT_d = nc.dram_tensor("biasT", [P, NEB], f32, kind="ExternalInput").ap()
    tw1_d = nc.dram_tensor("tw1", [T, E, H], f16, kind="ExternalInput").ap()
    tb1_d = nc.dram_tensor("tb1", [H, T], f32, kind="ExternalInput").ap()
    tw2_d = nc.dram_tensor("tw2", [H, T * P], f16, kind="ExternalInput").ap()
    out_d = nc.dram_tensor("out", [T, BC], f32, kind="ExternalOutput").ap()
    gbounce_d = nc.dram_tensor("gbounce", [TG, BC], f16, kind="Internal").ap()

    with tile.TileContext(nc) as tc:
        with (
            tc.tile_pool(name="const", bufs=1) as const,
            tc.tile_pool(name="exp", bufs=7) as exp_pool,
            tc.tile_pool(name="tmpv", bufs=2) as tmpv_pool,
        ):
            xt_t = [const.tile([P, BC], f16, tag=f"xt{k}", name=f"xt{k}") for k in range(KC)]
            wall_t = [const.tile([P, WALL], f16, tag=f"wall{k}", name=f"wall{k}") for k in range(KC)]
            biasT = const.tile([P, NEB], f32, tag="biasT", name="biasT")
            gates_sb = const.tile([TG, BC], f16, tag="gates_sb", name="gates_sb")
            gbc = const.tile([P, TG, BC], f16, tag="gbc", name="gbc")
            # acc[h]: [128, T, BC] fp16, t-major so paired ops hit both tasks
            acc = [
                const.tile([P, T, BC], f16, tag=f"acc{h}", name=f"acc{h}") for h in range(2)
            ]
            tw1_t = {}
            for t in range(T):
                for kc in range(2):
                    t_ = const.tile([P, H], f16, tag=f"tw1_{t}_{kc}", name=f"tw1_{t}_{kc}")
                    tw1_t[(t, kc)] = t_
            tb1 = const.tile([H, T], f32, tag="tb1", name="tb1")
            # tw2 replicated across 128 output partitions so tower2 fills a
            # full-width psum tile (cheap wide ACT drain instead of a
            # single-partition copy)
            tw2 = [
                const.tile([H, P], f16, tag=f"tw2_{t}", name=f"tw2_{t}") for t in range(T)
            ]
            hs_t = [
                const.tile([P, BC], f16, tag=f"hs{t}", name=f"hs{t}") for t in range(T)
            ]
            outf = [
                const.tile([P, BC], f32, tag=f"outf{t}", name=f"outf{t}")
                for t in range(T)
            ]

            # ---- input DMAs: head chunks first so eb0/eb1 start within
            # ~2us of boot; xt rest j-chunked for early gate availability.
            # gpsimd (slow SWDGE) assists with one xt chunk + late rows.
            HALF = NEB // 2 * P  # 1536 cols per half
            RW = {k: slice(k * P, (k + 1) * P) for k in range(KC)}
            nc.scalar.dma_start(biasT[:], biasT_d[:])
            nc.sync.dma_start(xt_t[0][:, 0:512], xt_d[RW[0], 0:512])
            nc.scalar.dma_start(xt_t[2][:, 0:512], xt_d[RW[2], 0:512])
            nc.sync.dma_start(wall_t[0][:, 0:256], wall_d[RW[0], 0:256])
            nc.scalar.dma_start(wall_t[2][:, 0:256], wall_d[RW[2], 0:256])
            nc.sync.dma_start(xt_t[1][:, 0:512], xt_d[RW[1], 0:512])
            nc.scalar.dma_start(xt_t[3][:, 0:512], xt_d[RW[3], 0:512])
            nc.sync.dma_start(wall_t[1][:, 0:256], wall_d[RW[1], 0:256])
            nc.scalar.dma_start(wall_t[3][:, 0:256], wall_d[RW[3], 0:256])
            # gate weights (tiny, needed by the gates GEMM right after eb0)
            for k in range(KC):
                nc.scalar.dma_start(wall_t[k][:, WCOLS:WALL], wall_d[RW[k], WCOLS:WALL])
            # rest of xt
            nc.sync.dma_start(xt_t[0][:, 512:BC], xt_d[RW[0], 512:BC])
            nc.scalar.dma_start(xt_t[2][:, 512:BC], xt_d[RW[2], 512:BC])
            nc.sync.dma_start(xt_t[1][:, 512:BC], xt_d[RW[1], 512:BC])
            nc.scalar.dma_start(xt_t[3][:, 512:BC], xt_d[RW[3], 512:BC])
            # rest of the first-half weights (gpsimd assists)
            nc.sync.dma_start(wall_t[0][:, 256:HALF], wall_d[RW[0], 256:HALF])
            nc.scalar.dma_start(wall_t[2][:, 256:HALF], wall_d[RW[2], 256:HALF])
            nc.gpsimd.dma_start(wall_t[1][:, 256:HALF], wall_d[RW[1], 256:HALF])
            nc.gpsimd.dma_start(wall_t[3][:, 256:HALF], wall_d[RW[3], 256:HALF])

            with tc.tile_pool(name="expps", bufs=2, space="PSUM") as expps_pool:
                exp_tiles = {}

                def expert_block(pos):
                    l = SWEEP[pos]
                    ps = expps_pool.tile([P, BC], f32, tag="expps", name="expps")
                    for k in range(KC):
                        for j in range(NBK):
                            cs = slice(j * 512, (j + 1) * 512)
                            nc.tensor.matmul(
                                ps[:, cs],
                                wall_t[k][:, pos * P : (pos + 1) * P],
                                xt_t[k][:, cs],
                                start=(k == 0),
                                stop=(k == KC - 1),
                                skip_group_check=True,
                            )
                    x_eb = exp_pool.tile([P, BC], f16, tag="xeb", name="xeb")
                    # last block drains in halves so the tail pipeline
                    # (combine -> tower) starts sooner
                    nh = 2 if pos == NEB - 1 else 1
                    for h2 in range(nh):
                        cs = slice(h2 * (BC // nh), (h2 + 1) * (BC // nh))
                        nc.scalar.activation(
                            x_eb[:, cs],
                            ps[:, cs],
                            mybir.ActivationFunctionType.Relu,
                            bias=biasT[:, l : l + 1],
                        )
                    exp_tiles[pos] = x_eb

                # first expert block warms PE while the rest of xt streams in
                expert_block(0)

                # gates in rows 0:16 of a psum-pool tile (k-outer for ldw-opt)
                gate_ps = expps_pool.tile([P, BC], f32, tag="expps", name="gate_ps")
                for k in range(KC):
                    for j in range(NBK):
                        cs = slice(j * 512, (j + 1) * 512)
                        nc.tensor.matmul(
                            gate_ps[0:TG, cs],
                            wall_t[k][:, WCOLS:WALL],
                            xt_t[k][:, cs],
                            start=(k == 0),
                            stop=(k == KC - 1),
                            skip_group_check=True,
                        )
                nc.scalar.copy(gates_sb[:], gate_ps[0:TG, :])

                # bounce gates through DRAM with partition-replicating reads.
                # Each queue writes its own row-slice then reads it back
                # (same-queue FIFO ordering guarantees write-before-read).
                # Rows ordered by first consumption in the sweep; scalar's
                # share is emitted inside the pos loop to keep the ACT
                # engine free for the first drains.
                def bounce(eng, rows):
                    for r in rows:
                        eng.dma_start(gbounce_d[r : r + 1, :], gates_sb[r : r + 1, :])
                    for r in rows:
                        eng.dma_start(
                            gbc[:, r, :], gbounce_d[r : r + 1, :].broadcast_to([P, BC])
                        )

                # wall second halves land well before sweep pos 12
                nc.scalar.dma_start(wall_t[2][:, HALF:WCOLS], wall_d[RW[2], HALF:WCOLS])
                nc.scalar.dma_start(wall_t[3][:, HALF:WCOLS], wall_d[RW[3], HALF:WCOLS])
                bounce(nc.sync, [0, 8])
                nc.sync.dma_start(wall_t[0][:, HALF:WCOLS], wall_d[RW[0], HALF:WCOLS])
                nc.sync.dma_start(wall_t[1][:, HALF:WCOLS], wall_d[RW[1], HALF:WCOLS])
                bounce(nc.sync, [1, 9])
                bounce(nc.gpsimd, [4, 5, 6, 7, 12, 13, 14, 15])

                # tower consts on the slow queue (needed only at the end)
                nc.gpsimd.dma_start(tb1[:], tb1_d[:])
                for t in range(T):
                    nc.gpsimd.dma_start(tw2[t][:], tw2_d[:, t * P : (t + 1) * P])
                    for kc in range(2):
                        nc.gpsimd.dma_start(
                            tw1_t[(t, kc)][:], tw1_d[t, kc * P : (kc + 1) * P, :]
                        )

                inited = set()

                def combine_block(pos):
                    l = SWEEP[pos]
                    expert, h = l // 2, l % 2
                    x_eb = exp_tiles.pop(pos)
                    a = acc[h]
                    if expert < S:
                        # shared expert: both tasks in one paired op
                        g2 = gbc[:, expert : expert + G + 1 : G, :]  # rows (g, 8+g)
                        xb = x_eb[:, None, :].broadcast_to([P, T, BC])
                        if ("s", h) not in inited:
                            inited.add(("s", h))
                            nc.vector.tensor_mul(a[:], xb, g2)
                        else:
                            tmp = tmpv_pool.tile([P, T, BC], f16, tag="tmp", name="tmp")
                            nc.vector.tensor_mul(tmp[:], xb, g2)
                            nc.vector.tensor_add(a[:], a[:], tmp[:])
                    else:
                        t = (expert - S) // K
                        j = t * G + S + (expert - S) % K
                        tmp = tmpv_pool.tile([P, T, BC], f16, tag="tmp", name="tmp")
                        nh = 2 if pos == NEB - 1 else 1
                        for h2 in range(nh):
                            cs = slice(h2 * (BC // nh), (h2 + 1) * (BC // nh))
                            nc.vector.tensor_mul(
                                tmp[:, 0, cs], x_eb[:, cs], gbc[:, j, cs]
                            )
                            nc.vector.tensor_add(
                                a[:, t, cs], a[:, t, cs], tmp[:, 0, cs]
                            )

                scalar_bounce = {1: [2, 10], 2: [3, 11]}
                for pos in range(NEB):
                    if pos > 0:
                        expert_block(pos)
                    if pos in scalar_bounce:
                        bounce(nc.scalar, scalar_bounce[pos])
                    combine_block(pos)

                # towers, j-chunked so the PE/ACT stages pipeline through
                # the tail (psum tiles from the same pool)
                for t in range(T):
                    hp = expps_pool.tile([P, BC], f32, tag="expps", name=f"hp{t}")
                    for kc in range(2):
                        for j in range(NBK):
                            cs = slice(j * 512, (j + 1) * 512)
                            nc.tensor.matmul(
                                hp[:, cs],
                                tw1_t[(t, kc)][:],
                                acc[kc][:, t, cs],
                                start=(kc == 0),
                                stop=(kc == 1),
                                skip_group_check=True,
                            )
                    for j in range(NBK):
                        cs = slice(j * 512, (j + 1) * 512)
                        nc.scalar.activation(
                            hs_t[t][:, cs],
                            hp[:, cs],
                            mybir.ActivationFunctionType.Relu,
                            bias=tb1[:, t : t + 1],
                        )
                for t in range(T):
                    op = expps_pool.tile([P, BC], f32, tag="expps", name=f"op{t}")
                    for j in range(NBK):
                        cs = slice(j * 512, (j + 1) * 512)
                        nc.tensor.matmul(
                            op[:, cs],
                            tw2[t][:],
                            hs_t[t][:, cs],
                            start=True,
                            stop=True,
                            skip_group_check=True,
                        )
                    for j in range(NBK):
                        cs = slice(j * 512, (j + 1) * 512)
                        nc.scalar.copy(outf[t][:, cs], op[:, cs])
                    nc.sync.dma_start(out_d[t : t + 1, :], outf[t][0:1, :])

    nc.compile()
    return nc


_NC = None


def _get_nc():
    global _NC
    if _NC is None:
        _NC = _build()
    return _NC


def _prep_shared(shared_W, shared_b, task_W, task_b, gate_W, tower_W1, tower_b1, tower_W2):
    # expert columns in logical order: shared 0..3, task (t, k)
    cols = [np.asarray(shared_W[s]) for s in range(S)]
    cols += [np.asarray(task_W[t, k]) for t in range(T) for k in range(K)]
    ecols = np.concatenate(cols, axis=1)  # [D, 3072], col c = expert*256 + e
    # rearrange 128-col blocks into sweep order
    blocks = ecols.reshape(D, NE * 2, P)
    swept = blocks[:, SWEEP, :].reshape(D, WCOLS)
    gwi = np.empty((D, TG), np.float32)
    for t in range(T):
        gwi[:, t * G : (t + 1) * G] = np.asarray(gate_W[t])  # col t*8+g
    wall = np.ascontiguousarray(
        np.concatenate([swept, gwi], axis=1), dtype=np.float16
    )
    bias_all = np.concatenate(
        [np.asarray(shared_b).reshape(-1), np.asarray(task_b).reshape(-1)]
    ).astype(np.float32)
    biasT = np.ascontiguousarray(bias_all.reshape(NEB, P).T)  # [128, 24], col = logical eb
    tw1 = np.ascontiguousarray(tower_W1, dtype=np.float16)
    tb1 = np.ascontiguousarray(np.asarray(tower_b1).T, dtype=np.float32)   # [H, T]
    w2 = np.asarray(tower_W2)[:, :, 0].T  # [H, T]
    tw2 = np.ascontiguousarray(np.repeat(w2, P, axis=1), dtype=np.float16)  # [H, T*P]
    return wall, biasT, tw1, tb1, tw2


def kernel(
    x,
    shared_W,
    shared_b,
    task_W,
    task_b,
    gate_W,
    tower_W1,
    tower_b1,
    tower_W2,
    tower_b2,
    _trace=False,
    _tmpdir=None,
):
    nc = _get_nc()
    x = np.asarray(x, dtype=np.float32)
    wall, biasT, tw1, tb1, tw2 = _prep_shared(
        shared_W, shared_b, task_W, task_b, gate_W, tower_W1, tower_b1, tower_W2
    )
    in_maps = []
    for c in range(NCORES):
        xt = np.ascontiguousarray(x[c * BC : (c + 1) * BC, :].T.astype(np.float16))
        in_maps.append(
            {
                "xt": xt,
                "wall": wall,
                "biasT": biasT,
                "tw1": tw1,
                "tb1": tb1,
                "tw2": tw2,
            }
        )
    kw = {}
    if _trace:
        kw = {"trace": True, "tmpdir": _tmpdir}
    res = run_bass_kernel_spmd(nc, in_maps, core_ids=list(range(NCORES)), **kw)
    out = np.concatenate([res.results[c]["out"] for c in range(NCORES)], axis=1)
    out = out + np.asarray(tower_b2, dtype=np.float32)[:, 0][:, None]
    result = out[:, :, None].astype(np.float32)  # [T, B, 1]
    if _trace:
        return result, res
    return result
